# revision 10
# baseline (speedup 1.0000x reference)
"""Bass/Trainium2 kernel for nn_BaselineLSTM (B=2048, T=512, H=128, twin=256).

v2 strategy (vs v1 baseline at 2.86ms):
  - Data-parallel: batch 2048 -> 8 cores x 256; each core runs 2 interleaved
    chunks of 128 batch (half-step offset hides per-step latency).
  - State transposed: hq/c2 = [H=128 partitions, batch free]. hq = h/2 and
    c2 = 2c so every nonlinearity is a plain Sigmoid/Tanh with free affine
    pre-scale, and all fixups ride on fused DVE scalar_tensor_tensor ops.
  - Gate order in PSUM: [g|i|f|o]. The g rows of all weight matrices are
    pre-scaled x2 on the host so ONE merged Sigmoid over all 4 gates gives
    sigma(i,f,o) and sigma(2g) = (tanh(g)+1)/2. That cuts ACT from 3 to 2
    instructions per chunk-step:
        sig = Sigmoid(gates)              # FD=512, PSUM src
        tc  = Tanh(0.5 * c2)              # FD=128 = tanh(c)
  - DVE (4 fused ops per chunk-step, all bf16 2x mode):
        uh = (sig_g - 0.5) * sig_i        #  = tanh(g)*sigma(i)/2
        t2 = sig_f * c2                   #  = 2*sigma(f)*c
        c2 = (uh * 4) + t2                #  = 2*c_new
        hq = (tc * 0.5) * sig_o           #  = h_new/2
  - W_hh (and phase-H folded W_hh + W_ih@W_out) are scaled x2 on the host to
    compensate hq = h/2; W_out x2 likewise.
  - Phase P input+bias: per-gate K=2 matmuls vs packed [y_t; 1] rows
    (emitted BEFORE the gate matmuls so they run off the critical path).
  - Phase H: input is folded into the recurrence; bias enters via ONE K=4
    matmul vs a constant block-diagonal ones tile (FD=512).
  - Predictions: h ring + shifted-W_out batched matmul every 4 steps, PSUM
    accumulated 128 steps, flushed via Vector copy + DMA (as v1); b_out is
    added on the host.
"""

import functools

import ml_dtypes
import numpy as np

import concourse.bacc as bacc
import concourse.tile as tile
from concourse import mybir
from concourse.bass_utils import run_bass_kernel_spmd

F32 = mybir.dt.float32
BF16 = mybir.dt.bfloat16
AF = mybir.ActivationFunctionType
OP = mybir.AluOpType

H = 128          # hidden
NCORES = 8
BS = 256         # batch per core
BC = 128         # batch per chunk
NCHUNK = 2

# pytorch gate order (i, f, g, o) -> kernel order (g, i, f, o)
_PERM = np.concatenate([np.arange(256, 384), np.arange(0, 128),
                        np.arange(128, 256), np.arange(384, 512)])
# sigma-trick row scale: x2 on the g rows (kernel order block 0)
_SROW = np.concatenate([np.full(128, 2.0), np.ones(384)]).astype(np.float32)


def _build_body(tc, d, NP, NH):
    nc = tc.nc
    NT = NP + NH

    import contextlib
    with contextlib.ExitStack() as ctx:
        consts = ctx.enter_context(tc.tile_pool(name="consts", bufs=1))
        state = ctx.enter_context(tc.tile_pool(name="state", bufs=1))
        work = ctx.enter_context(tc.tile_pool(name="work", bufs=3))
        gpool = ctx.enter_context(tc.tile_pool(name="gates", bufs=3, space="PSUM"))
        ppool = ctx.enter_context(tc.tile_pool(name="ppsum", bufs=1, space="PSUM"))

        # ---- constants to SBUF
        whhT_p = consts.tile([H, 4 * H], BF16, tag="whhT_p")
        whhT_h = consts.tile([H, 4 * H], BF16, tag="whhT_h")
        lp8 = consts.tile([8, H], BF16, tag="lp8")
        lh4 = consts.tile([4, H], BF16, tag="lh4")
        ones4 = consts.tile([4, 4 * H], BF16, tag="ones4")
        woutZ = consts.tile([H, 2 * H], BF16, tag="woutZ")
        nc.sync.dma_start(out=whhT_p, in_=d["whhT_p"])
        nc.sync.dma_start(out=whhT_h, in_=d["whhT_h"])
        nc.sync.dma_start(out=lp8, in_=d["lp8"])
        nc.sync.dma_start(out=lh4, in_=d["lh4"])
        nc.sync.dma_start(out=ones4, in_=d["ones4"])
        nc.sync.dma_start(out=woutZ, in_=d["woutZ"])
        xpool = ctx.enter_context(tc.tile_pool(name="xq8", bufs=4))

        # ---- state: hq kept in a 4-slot ring (slot s%4) for batched preds.
        hist = []
        c2 = []
        for ch in range(NCHUNK):
            hh = state.tile([H, 4 * BC], BF16, tag=f"hist{ch}")
            c = state.tile([H, BC], BF16, tag=f"c2_{ch}")
            nc.vector.memset(hh, 0.0)
            nc.vector.memset(c, 0.0)
            hist.append(hh)
            c2.append(c)

        pps = [None, None]
        sigs = [None, None]
        gates_l = [None, None]

        def front(s, ch):
            """Input/bias + gate matmuls + merged sigmoid for step s."""
            phase_p = s < NP
            gates = gpool.tile([H, 4 * H], F32, tag=f"g{ch}",
                               name=f"g{ch}_{s}")
            gates_l[ch] = gates
            whh = whhT_p if phase_p else whhT_h
            hprev = hist[ch][:, ((s - 1) % 4) * BC: ((s - 1) % 4 + 1) * BC]

            # input/bias opener MM writes the FULL bank (start=True), then each
            # gate block gets exactly ONE accumulating matmul. Any other
            # start/stop shape (interleaved open groups, double accumulates on
            # an element) computes garbage on HW.
            if phase_p:
                xb = xpool.tile([8, 4 * H], BF16, tag=f"xq8_{ch}",
                                name=f"xq8_{ch}_{s}")
                nc.sync.dma_start(out=xb, in_=d["xq8"][s * NCHUNK + ch])
                nc.tensor.matmul(gates, lp8, xb, start=True, stop=False,
                                 skip_group_check=True)
            else:
                nc.tensor.matmul(gates, lh4, ones4, start=True, stop=False,
                                 skip_group_check=True)
            # gate matmuls accumulate on top
            for k in range(4):
                nc.tensor.matmul(gates[:, k * H:(k + 1) * H],
                                 whh[:, k * H:(k + 1) * H], hprev,
                                 start=False, stop=True,
                                 skip_group_check=True)
            # f32 output: the g-block rides a half-range signal around 0.5
            # ((sig-0.5) recovers tanh); bf16 output would inject 2^-9
            # absolute noise that the recurrence amplifies past tolerance.
            # Split g|i|f vs o: the c2 chain only needs g,i,f, and the short
            # sigma_o slot reduces strict-FIFO blocking of the other chunk's
            # Tanh on the scalar engine.
            sig = work.tile([H, 4 * H], F32, tag=f"sig{ch}",
                            name=f"sig{ch}_{s}")
            nc.scalar.activation(sig[:, 0:3 * H], gates[:, 0:3 * H],
                                 AF.Sigmoid)
            nc.scalar.activation(sig[:, 3 * H:4 * H], gates[:, 3 * H:4 * H],
                                 AF.Sigmoid)
            sigs[ch] = sig

        def back(s, ch):
            """c2/hq update for step s + batched prediction matmul."""
            sig = sigs[ch]
            t2 = work.tile([H, BC], BF16, tag=f"t2{ch}", name=f"t2{ch}_{s}")
            nc.gpsimd.tensor_mul(t2, sig[:, 2 * H:3 * H], c2[ch])
            uh = work.tile([H, BC], BF16, tag=f"uh{ch}", name=f"uh{ch}_{s}")
            nc.vector.scalar_tensor_tensor(
                uh, sig[:, 0:H], 0.5, sig[:, H:2 * H],
                OP.subtract, OP.mult)
            nc.vector.scalar_tensor_tensor(
                c2[ch], uh, 4.0, t2, OP.mult, OP.add)
            tcn = work.tile([H, BC], BF16, tag=f"tcn{ch}", name=f"tcn{ch}_{s}")
            nc.scalar.activation(tcn, c2[ch], AF.Tanh, scale=0.5)
            hslot = hist[ch][:, (s % 4) * BC: (s % 4 + 1) * BC]
            nc.vector.scalar_tensor_tensor(
                hslot, tcn, 0.5, sig[:, 3 * H:4 * H], OP.mult, OP.mult)

            # Predictions: every 4 steps, p for steps 4G..4G+3 = one matmul
            # W_out @ [h_0|h_1|h_2|h_3]; row placement via shifted zero-pad.
            if s % 4 == 3 or s == NT - 1:
                G = s // 4
                r = G % 32
                n = (s % 4 + 1) * BC
                if r == 0:
                    pps[ch] = ppool.tile([H, 4 * BC], F32, tag=f"pps{ch}",
                                         name=f"pps{ch}_{s}")
                nc.tensor.matmul(pps[ch][:, 0:n],
                                 woutZ[:, H - r: 2 * H - r],
                                 hist[ch][:, 0:n],
                                 start=(r == 0), stop=(r == 31 or s == NT - 1),
                                 skip_group_check=True)
                if r == 31 or s == NT - 1:
                    e = G // 32
                    pc = work.tile([32, 4 * BC], F32, tag=f"pc{ch}",
                                   name=f"pc{ch}_{s}")
                    nc.vector.tensor_copy(pc, pps[ch][0:32, :])
                    nc.sync.dma_start(out=d["preds"][e, ch], in_=pc)

        # Software pipeline: chunk 1 runs half a step behind chunk 0.
        for s in range(NT):
            front(s, 0)
            if s > 0:
                back(s - 1, 1)
            front(s, 1)
            back(s, 0)
        back(NT - 1, 1)


@functools.lru_cache(maxsize=2)
def _program(NP, NH):
    nc = bacc.Bacc("TRN2", target_bir_lowering=False, debug=False,
                   num_devices=NCORES)
    NT = NP + NH
    NEP = (NT + 127) // 128
    d = {
        "whhT_p": nc.dram_tensor("whhT_p", [H, 4 * H], BF16,
                                 kind="ExternalInput").ap(),
        "whhT_h": nc.dram_tensor("whhT_h", [H, 4 * H], BF16,
                                 kind="ExternalInput").ap(),
        "lp8": nc.dram_tensor("lp8", [8, H], BF16, kind="ExternalInput").ap(),
        "lh4": nc.dram_tensor("lh4", [4, H], BF16, kind="ExternalInput").ap(),
        "ones4": nc.dram_tensor("ones4", [4, 4 * H], BF16,
                                kind="ExternalInput").ap(),
        "woutZ": nc.dram_tensor("woutZ", [H, 2 * H], BF16,
                                kind="ExternalInput").ap(),
        "xq8": nc.dram_tensor("xq8", [NP * NCHUNK, 8, 4 * H], BF16,
                              kind="ExternalInput").ap(),
        "preds": nc.dram_tensor("preds", [NEP, NCHUNK, 32, 4 * BC], F32,
                                kind="ExternalOutput").ap(),
    }
    with tile.TileContext(nc) as tc:
        _build_body(tc, d, NP, NH)
    nc.compile()
    return nc


def _host_prep(y_flow, W_ih, W_hh, b_ih, b_hh, W_out, b_out, NP):
    """Build per-core input maps. y_flow: (B, T, 1) f32."""
    bf = ml_dtypes.bfloat16
    W_ih = np.asarray(W_ih, np.float32)
    W_hh = np.asarray(W_hh, np.float32)
    W_out = np.asarray(W_out, np.float32)
    bias = np.asarray(b_ih, np.float32) + np.asarray(b_hh, np.float32)
    b_out = np.asarray(b_out, np.float32)

    W_hh_H = W_hh + W_ih @ W_out          # [4H, H]
    bias_H = bias + W_ih[:, 0] * b_out[0]

    # sigma-trick x2 on g rows; x2 everywhere to compensate hq = h/2.
    sc = _SROW[:, None]
    whhT_p = np.ascontiguousarray((2.0 * sc * W_hh[_PERM]).T).astype(bf)
    whhT_h = np.ascontiguousarray((2.0 * sc * W_hh_H[_PERM]).T).astype(bf)
    # phase-P input/bias: K=8 opener. lp8 rows 0-3 = per-gate W_ih columns,
    # rows 4-7 = per-gate bias; xq8 pairs them with [x-blockdiag; 1-blockdiag].
    lp8 = np.concatenate([(_SROW * W_ih[_PERM, 0]).reshape(4, H),
                          (_SROW * bias[_PERM]).reshape(4, H)]).astype(bf)
    # phase-H bias: K=4 vs constant block-diagonal ones
    lh4 = (_SROW * bias_H[_PERM]).reshape(4, H).astype(bf)
    ones4 = np.zeros((4, 4 * H), np.float32)
    for k in range(4):
        ones4[k, k * H:(k + 1) * H] = 1.0
    ones4 = ones4.astype(bf)
    woutZ = np.zeros((H, 2 * H), np.float32)                      # [H, 256]
    woutZ[:, H] = 2.0 * W_out[0]
    woutZ = woutZ.astype(bf)

    y = np.asarray(y_flow, np.float32)[:, :, 0]                   # [B, T]
    B = y.shape[0]
    in_maps = []
    for core in range(NCORES):
        yc = y[core * BS:(core + 1) * BS]                         # [BS, T]
        # xq8[g=(s*2+ch), j, k*H+b]: j<4: x[b]*d(j==k); j>=4: d(j-4==k)
        xq8 = np.zeros((NP, NCHUNK, 8, 4 * H), np.float32)
        yv = yc[:, :NP].T.reshape(NP, NCHUNK, BC)    # [s, ch, b]
        for k in range(4):
            xq8[:, :, k, k * H:(k + 1) * H] = yv
            xq8[:, :, 4 + k, k * H:(k + 1) * H] = 1.0
        in_maps.append({
            "whhT_p": whhT_p, "whhT_h": whhT_h, "lp8": lp8, "lh4": lh4,
            "ones4": ones4, "woutZ": woutZ,
            "xq8": xq8.reshape(NP * NCHUNK, 8, 4 * H).astype(bf),
        })
    return in_maps


def kernel(y_flow, x_dyn, W_ih, W_hh, b_ih, b_hh, W_out, b_out, twin_idx,
           _trace=False):
    twin = int(twin_idx)
    assert twin == 256, f"kernel hardcodes twin_idx=256, got {twin}"
    B, T, _ = y_flow.shape
    assert (B, T) == (2048, 512)
    NP, NH = twin - 1, T - twin
    NT = NP + NH

    nc = _program(NP, NH)
    in_maps = _host_prep(y_flow, W_ih, W_hh, b_ih, b_hh, W_out, b_out, NP)
    res = run_bass_kernel_spmd(nc, in_maps, core_ids=list(range(NCORES)),
                               trace=_trace)

    b_out = np.asarray(b_out, np.float32)
    out = np.empty((B, NT, 1), np.float32)
    for core in range(NCORES):
        p = np.asarray(res.results[core]["preds"], np.float32)
        nep = p.shape[0]
        a = p.reshape(nep, NCHUNK, 32, 4, BC)      # [e, ch, r, j, b]
        for ch in range(NCHUNK):
            blk = a[:, ch].transpose(3, 0, 1, 2).reshape(BC, -1)[:, :NT]
            out[core * BS + ch * BC: core * BS + (ch + 1) * BC, :, 0] = \
                blk + b_out[0]
    if _trace:
        kernel._last_results = res
    return out
